# revision 66
# baseline (speedup 1.0000x reference)
"""Trainium2 Bass kernel for nn_CAAN_78323023610440.

Reference computation (per batch b):
    q = x @ Wq.T + bq;  k = x @ Wk.T + bk;  v = x @ Wv.T + bv
    beta = softmax(q @ k.T / sqrt(D), axis=-1)
    final = (beta @ v) @ Ww.T + bw            # [B, N]

Algebraic restructuring (exact, modulo fp reassociation):
  *  q·k = x A x^T + r[n] + c[m] + const, with A = Wq^T Wk,
     r[n] = x[n]·(Wq^T bk) (row-constant -> drops out of softmax),
     c[m] = x[m]·(Wk^T bq) (key-side constant, kept).
  *  c[m] = x[m]·g shares the phase-2 contraction structure, so it is
     folded into TT during the phase-1 PSUM->SBUF cast: TT'[e,n] =
     TT[e,n] + g[e]  =>  sum_e TT'[e,n] x[m,e] = s[n,m] + c[m]. No exp
     bias needed at all.
  *  (beta @ v) @ Ww^T = beta @ (x @ (Wv^T Ww^T) + bv·Ww) -> the whole
     V projection collapses into a per-key scalar wv[m].
  *  final[n] = sum_m exp(l[n,m]) wv[m] / sum_m exp(l[n,m]) + bw
     (softmax max-subtraction skipped: logits are O(1) here, exp is safe
      in fp32 — both sums are formed unnormalized and divided at the end).

Sharding: 8 cores = 4 batches x 2 query-halves. Each core, for its 1024
queries n and all 2048 keys m of its batch:
    TT'[e, n] = sum_f A[f, e] xT[f, n] + g[e]     (phase 1, PE + DVE cast)
    S[n, m]   = sum_e TT'[e, n] xT[e, m]          (phase 2, queries on
                                                   partitions, 8 n-slices
                                                   x 4 key-chunks)
    P         = Exp(S/32)                         (ScalarE, no bias)
    num[n,c]  = sum_m P[n, m] wn[m]               (GpSimd mult + DVE reduce)
    den[n,c]  = sum_m P[n, m]                     (accum_out on the exp)
Host sums the 4 chunk partials, divides, adds bw. Keys are passed to
each core local-half-first so all 8 cores run an identical program.

With queries on partitions the old per-key-tile reduction matmuls (32 x
~216ns of pure PE time) disappear; the PE runs only the 384 compute
matmuls. All operands bf16 (FWL weight loads, ~216ns/512-col matmul).

Measured (NTFF, core 0): ~102.7-103.6us HW exec (prior session's
f32r/orientation-A baseline: 118.8-140us), rel err 1.50e-3 vs the 2e-2
gate. Budget: ~7.5us fixed preamble, warm-up matmuls to ~10.5us (first
input DMAs land), 384 MMs at 216ns + ~3.5us of instruction-ring wraps
and cold-clock ramp, ~3.4us exp->mult->reduce->DMA tail, ~3us end
barrier. Known dead ends: tensor_tensor_reduce (HW crash), fp8
DoubleRow (est rel err ~2e-2, no margin), col-tiled reductions (wash),
splitting the last S-group (net +0.5us), splitting f=0 input DMAs finer
than halves (extra issue instructions delay the whole DMA program by
more than the first matmul gains), split-start PSUM groups (CoreSim
cannot model them; HW first_mm semantics uncertain).
"""

import numpy as np

import ml_dtypes

import concourse.tile as tile
from concourse import bacc, mybir
from concourse.bass_utils import run_bass_kernel_spmd
from contextlib import ExitStack

B = 4
N = 2048
D = 1024
P = 128
ET = D // P          # 8 contraction tiles over D
NQ = N // 2          # 1024 local queries per core
CHUNK = 512          # PSUM bank limit (512 fp32 outputs)
NCH = NQ // CHUNK    # 2 query chunks (phase 1)
KCH = N // CHUNK     # 4 key chunks (phase 2)
SCALE = 0.03125      # 1/sqrt(D), exact
WARMUP_MM = 6        # dummy matmuls to lift the PE HAM clock-gate early
F32 = mybir.dt.float32
BF16 = mybir.dt.bfloat16
EXP = mybir.ActivationFunctionType.Exp
ADD = mybir.AluOpType.add
MULT = mybir.AluOpType.mult
AXF = mybir.AxisListType.X

_CACHE = {}


def _build():
    mdt = BF16
    nc = bacc.Bacc(
        "TRN2",
        target_bir_lowering=False,
        debug=False,
        enable_asserts=False,
        num_devices=8,
    )
    # Per-core inputs. xq = x[b, local half].T ; xk2 = x[b, other half].T
    # (keys ordered local-first so the program is core-independent).
    xq_d = nc.dram_tensor("xq", [D, NQ], mdt, kind="ExternalInput")
    xk2_d = nc.dram_tensor("xk2", [D, NQ], mdt, kind="ExternalInput")
    a_d = nc.dram_tensor("A", [D, D], mdt, kind="ExternalInput")
    g_d = nc.dram_tensor("g", [P, ET], F32, kind="ExternalInput")
    wn_d = nc.dram_tensor("wn", [P, KCH, CHUNK], mdt, kind="ExternalInput")
    num_d = nc.dram_tensor("num", [P, KCH, ET], F32, kind="ExternalOutput")
    den_d = nc.dram_tensor("den", [P, KCH, ET], F32, kind="ExternalOutput")

    with tile.TileContext(nc) as tc, ExitStack() as ctx:
        const = ctx.enter_context(tc.tile_pool(name="const", bufs=1))
        ptp = ctx.enter_context(tc.tile_pool(name="pt", bufs=6))
        tmpp = ctx.enter_context(tc.tile_pool(name="tmp", bufs=3))
        workp = ctx.enter_context(
            tc.tile_pool(name="psum_work", bufs=7, space="PSUM")
        )
        wup = ctx.enter_context(
            tc.tile_pool(name="psum_wu", bufs=1, space="PSUM")
        )

        xq_sb = const.tile([P, ET, NQ], mdt)    # [p, f, n] : xT local cols
        xk2_sb = const.tile([P, ET, NQ], mdt)   # [p, f, n] : xT other cols
        a_sb = const.tile([P, ET, D], mdt)      # [p, f, e] : A tiles
        tt_sb = const.tile([P, ET, NQ], mdt)    # [p, e, n] : TT' tiles
        g_sb = const.tile([P, ET], F32)         # g[e] fold for the TT cast
        wn_sb = const.tile([P, KCH, CHUNK], mdt)  # wv[m] replicated per row
        num_sb = const.tile([P, KCH, ET], F32)
        den_sb = const.tile([P, KCH, ET], F32)
        wu_sb = const.tile([P, CHUNK], BF16)    # warmup operand (garbage ok)
        wu_sink = const.tile([P, 1], F32)

        # PE warm-up: keep TensorE busy from t~0 so the HAM clock-gate
        # lifts to 8/8 before the real matmuls start (they are DMA-gated),
        # and fill the DMA-paced holes of the first TT block below.
        # Operand contents are irrelevant.
        nc.gpsimd.memset(wu_sb[:], 0.0)
        wu_ps = wup.tile([P, CHUNK], F32)
        # per-f filler count topping up the real matmuls one (A[f], xq[f])
        # tile-pair arrival enables in the first block. Zero since the
        # j-outer block-0 ordering: the fillers burned ~1.7us of PE
        # unconditionally, while a (now-rare) DMA-late stall costs the
        # same only when it actually happens.
        wpf = 0
        n_wu = WARMUP_MM + wpf * ET
        wu_iter = iter(range(n_wu))

        def warm(k):
            for _ in range(k):
                w = next(wu_iter, None)
                if w is None:
                    return
                nc.tensor.matmul(
                    wu_ps[:],
                    wu_sb[:, :P],
                    wu_sb[:],
                    start=(w == 0),
                    stop=(w == n_wu - 1),
                )

        warm(WARMUP_MM)

        # Input DMAs. Phase-1 block 0 (e 0-2) needs only A columns 0:384,
        # so those stream first alongside xq — this shrinks the critical
        # bytes gating the first matmul block. The rest of A arrives while
        # block 0 computes; wn is first read at the first phase-2 exp and
        # xk2 only gates key chunks 2-3 of phase 2, so they stream last.
        # DMA issue instructions cost ~750ns each on the issuing engine's
        # HWDGE ring; alternating Sync/Scalar rings halves the issue-cadence
        # latency (Scalar is idle until the first exp at ~45us).
        # f=0 is split finer: the very first matmul is gated on
        # A[:,0,0:128] + xq[:,0,0:512] (161KB) instead of the full 354KB
        # pair (the first transfers share DMA bandwidth round-robin with
        # everything issued behind them).
        E0 = 3 * P
        nc.sync.dma_start(a_sb[:, 0, :P], a_d[0:P, :P])
        nc.scalar.dma_start(xq_sb[:, 0, :CHUNK], xq_d[0:P, :CHUNK])
        nc.sync.dma_start(a_sb[:, 0, P:E0], a_d[0:P, P:E0])
        nc.scalar.dma_start(xq_sb[:, 0, CHUNK:], xq_d[0:P, CHUNK:])
        for f in range(1, ET):
            nc.sync.dma_start(a_sb[:, f, :E0], a_d[f * P:(f + 1) * P, :E0])
            nc.scalar.dma_start(xq_sb[:, f, :], xq_d[f * P:(f + 1) * P, :])
        nc.sync.dma_start(g_sb[:], g_d[:])
        # Keep these as per-f DMA instructions: separate instructions fan
        # out across parallel HW DMA queues (consolidating them into one
        # strided DMA measured ~6us slower end-to-end).
        for f in range(ET):
            eng = nc.sync if f % 2 == 0 else nc.scalar
            eng.dma_start(a_sb[:, f, E0:], a_d[f * P:(f + 1) * P, E0:])
        nc.sync.dma_start(wn_sb[:, 0:2, :], wn_d[:, 0:2, :])
        nc.scalar.dma_start(wn_sb[:, 2:4, :], wn_d[:, 2:4, :])
        for f in range(ET):
            eng = nc.sync if f % 2 == 0 else nc.scalar
            eng.dma_start(xk2_sb[:, f, :], xk2_d[f * P:(f + 1) * P, :])

        # Phase 1: TT'[e, n] = sum_f A[f, e-cols]^T . xT[f, n]  (+ g[e] in
        # the cast). e-blocks of 3 keep 6 PSUM accumulation groups open so
        # each arriving (A[f], xq[f]) DMA pair feeds 6 matmuls (less PE
        # starvation while inputs stream in).
        BLOCKS = [(0, 3), (3, 3), (6, 2)]
        for eb, (e0, blk) in enumerate(BLOCKS):
            pss = []
            for el in range(blk):
                row = [
                    workp.tile([P, CHUNK], F32,
                               name=f"tt_ps_{eb}_{el}_{j}", tag="ps")
                    for j in range(NCH)
                ]
                pss.append(row)
            for f in range(ET):
                # Block 0 runs j-outer: the three j=0 matmuls (first xq
                # half) front-load ~1.3us of cold-pace work, matching the
                # second xq half's later DMA arrival (it was the ~0.9us
                # stall at the start of the real stream).
                order = (
                    [(el, j) for j in range(NCH) for el in range(blk)]
                    if eb == 0 else
                    [(el, j) for el in range(blk) for j in range(NCH)]
                )
                for el, j in order:
                    e = e0 + el
                    nc.tensor.matmul(
                        pss[el][j][:],
                        a_sb[:, f, e * P:(e + 1) * P],
                        xq_sb[:, f, j * CHUNK:(j + 1) * CHUNK],
                        start=(f == 0),
                        stop=(f == ET - 1),
                    )
                if eb == 0:
                    # absorb the DMA-arrival pacing of the first block
                    warm(wpf)
            for el in range(blk):
                e = e0 + el
                for j in range(NCH):
                    nc.vector.tensor_scalar_add(
                        tt_sb[:, e, j * CHUNK:(j + 1) * CHUNK],
                        pss[el][j][:],
                        g_sb[:, e:e + 1],
                    )
            if eb == 0:
                warm(100)  # flush any leftover warmups
                nc.vector.tensor_copy(wu_sink[:], wu_ps[:, :1])

        # Phase 2: per (key-chunk c, query-slice s): S, exp, and the two
        # free-axis reductions on DVE (num's multiply on GpSimd). Key
        # chunks 0-1 (local xq keys) run first so the xk2 stream has until
        # ~mid-phase-2 to land.
        for c in range(KCH):
            xsrc = xq_sb if c < 2 else xk2_sb
            off = (c % 2) * CHUNK
            for s in range(ET):
                ps = workp.tile([P, CHUNK], F32, name=f"s_ps_{c}_{s}",
                                tag="ps")
                for e in range(ET):
                    nc.tensor.matmul(
                        ps[:],
                        tt_sb[:, e, s * P:(s + 1) * P],
                        xsrc[:, e, off:off + CHUNK],
                        start=(e == 0),
                        stop=(e == ET - 1),
                    )
                pt = ptp.tile([P, CHUNK], mdt, name=f"pt_{c}_{s}", tag="pt")
                # den rides the exp itself: ScalarE reduces its own output
                # into a per-partition scalar (softmax-denominator pattern),
                # halving the DVE load.
                nc.scalar.activation(
                    pt[:], ps[:], EXP, scale=SCALE,
                    accum_out=den_sb[:, c, s:s + 1],
                )
                tmp = tmpp.tile([P, CHUNK], F32, name=f"tmp_{c}_{s}",
                                tag="tmp")
                # GpSimd takes the num multiply except on the final two
                # groups, where the (slower, serialized) GpSimd would sit
                # on the exec tail; DVE has the slack there.
                meng = nc.vector if c == KCH - 1 and s >= ET - 2 else nc.gpsimd
                meng.tensor_tensor(tmp[:], pt[:], wn_sb[:, c, :], MULT)
                nc.vector.tensor_reduce(
                    num_sb[:, c, s:s + 1], tmp[:], AXF, ADD
                )

        # num in two pieces: chunks 0-2 stream out ~14us before the end;
        # only the tiny c=3 piece's completion sits inside the end barrier.
        nc.sync.dma_start(num_d[:, 0:KCH - 1, :], num_sb[:, 0:KCH - 1, :])
        nc.scalar.dma_start(den_d[:], den_sb[:])
        nc.sync.dma_start(num_d[:, KCH - 1:KCH, :],
                          num_sb[:, KCH - 1:KCH, :])

    nc.compile()
    return nc


def _get_nc():
    if "nc" not in _CACHE:
        _CACHE["nc"] = _build()
    return _CACHE["nc"]


def _prep(x, Wq, bq, Wk, bk, Wv, bv, Ww, bw):
    """Host-side sharding + weight folding -> per-core input maps."""
    x = np.asarray(x, dtype=np.float32)
    Wq = np.asarray(Wq, dtype=np.float32)
    bq = np.asarray(bq, dtype=np.float32)
    Wk = np.asarray(Wk, dtype=np.float32)
    Wv = np.asarray(Wv, dtype=np.float32)
    bv = np.asarray(bv, dtype=np.float32)
    Ww = np.asarray(Ww, dtype=np.float32)
    idt = ml_dtypes.bfloat16

    # Host-side weight folding (cheap: one 1024^3 sgemm + matvecs).
    A = np.ascontiguousarray(Wq.T @ Wk).astype(idt)  # [f, e]
    g = Wk.T @ bq                                    # key-side logit constant
    wv_eff = Wv.T @ Ww[0]                            # collapsed V @ Ww^T
    cvw = float(bv @ Ww[0])

    wv_all = x @ wv_eff + cvw                        # [B, N]
    g_arr = np.ascontiguousarray(g.reshape(ET, P).T.astype(np.float32))

    in_maps = []
    for core in range(8):
        b, h = divmod(core, 2)
        lo = np.arange(h * NQ, (h + 1) * NQ)
        hi = np.arange((1 - h) * NQ, (2 - h) * NQ)
        order = np.concatenate([lo, hi])             # keys: local half first
        wv_ord = wv_all[b][order].astype(idt)        # [N]
        wn = np.ascontiguousarray(
            np.broadcast_to(wv_ord.reshape(1, KCH, CHUNK), (P, KCH, CHUNK))
        )
        in_maps.append(
            {
                "xq": np.ascontiguousarray(x[b, lo].T.astype(idt)),
                "xk2": np.ascontiguousarray(x[b, hi].T.astype(idt)),
                "A": A,
                "g": g_arr,
                "wn": wn,
            }
        )
    return in_maps


def _gather(res, bw):
    bw = np.asarray(bw, dtype=np.float32)
    final = np.empty((B, N), dtype=np.float32)
    for core in range(8):
        b, h = divmod(core, 2)
        num = res.results[core]["num"].sum(axis=1)   # [P, ET]
        den = res.results[core]["den"].sum(axis=1)
        final[b, h * NQ:(h + 1) * NQ] = (num / den).T.reshape(NQ) + bw[0]
    return final


def kernel(x, Wq, bq, Wk, bk, Wv, bv, Ww, bw):
    nc = _get_nc()
    in_maps = _prep(x, Wq, bq, Wk, bk, Wv, bv, Ww, bw)
    res = run_bass_kernel_spmd(nc, in_maps, core_ids=list(range(8)))
    return _gather(res, bw)


def run_profiled(inputs, trace_cores=(0,)):
    """Run once with NTFF profiling; returns BassKernelResults."""
    nc = _get_nc()
    in_maps = _prep(**inputs)
    res = run_bass_kernel_spmd(
        nc, in_maps, core_ids=list(range(8)), trace=True,
        trace_cores=list(trace_cores),
    )
    return res


# revision 67
# speedup vs baseline: 1.1699x; 1.1699x over previous
"""Trainium2 Bass kernel for nn_CAAN_78323023610440.

Reference computation (per batch b):
    q = x @ Wq.T + bq;  k = x @ Wk.T + bk;  v = x @ Wv.T + bv
    beta = softmax(q @ k.T / sqrt(D), axis=-1)
    final = (beta @ v) @ Ww.T + bw            # [B, N]

Algebraic restructuring (exact, modulo fp reassociation):
  *  q·k = x A x^T + r[n] + c[m] + const, with A = Wq^T Wk,
     r[n] = x[n]·(Wq^T bk) (row-constant -> drops out of softmax),
     c[m] = x[m]·(Wk^T bq) (key-side constant, kept).
  *  c[m] = x[m]·g shares the phase-2 contraction structure, so it is
     folded into TT during the phase-1 PSUM->SBUF cast: TT'[e,n] =
     TT[e,n] + g[e]  =>  sum_e TT'[e,n] x[m,e] = s[n,m] + c[m]. No exp
     bias needed at all.
  *  (beta @ v) @ Ww^T = beta @ (x @ (Wv^T Ww^T) + bv·Ww) -> the whole
     V projection collapses into a per-key scalar wv[m].
  *  final[n] = sum_m exp(l[n,m]) wv[m] / sum_m exp(l[n,m]) + bw
     (softmax max-subtraction skipped: logits are O(1) here, exp is safe
      in fp32 — both sums are formed unnormalized and divided at the end).

Sharding: 8 cores = 4 batches x 2 query-halves. Each core, for its 1024
queries n and all 2048 keys m of its batch:
    TT'[e, n] = sum_f A[f, e] xT[f, n] + g[e]     (phase 1, PE + DVE cast)
    S[n, m]   = sum_e TT'[e, n] xT[e, m]          (phase 2, queries on
                                                   partitions, 8 n-slices
                                                   x 4 key-chunks)
    P         = Exp(S/32)                         (ScalarE, no bias)
    num[n,c]  = sum_m P[n, m] wn[m]               (GpSimd mult + DVE reduce)
    den[n,c]  = sum_m P[n, m]                     (accum_out on the exp)
Host sums the 4 chunk partials, divides, adds bw. Keys are passed to
each core local-half-first so all 8 cores run an identical program.

With queries on partitions the old per-key-tile reduction matmuls (32 x
~216ns of pure PE time) disappear; the PE runs only the 384 compute
matmuls. All operands bf16 (FWL weight loads, ~216ns/512-col matmul).

Measured (NTFF, core 0): ~102.7-103.6us HW exec (prior session's
f32r/orientation-A baseline: 118.8-140us), rel err 1.50e-3 vs the 2e-2
gate. Budget: ~7.5us fixed preamble, warm-up matmuls to ~10.5us (first
input DMAs land), 384 MMs at 216ns + ~3.5us of instruction-ring wraps
and cold-clock ramp, ~3.4us exp->mult->reduce->DMA tail, ~3us end
barrier. Known dead ends: tensor_tensor_reduce (HW crash), fp8
DoubleRow (est rel err ~2e-2, no margin), col-tiled reductions (wash),
splitting the last S-group (net +0.5us), splitting f=0 input DMAs finer
than halves (extra issue instructions delay the whole DMA program by
more than the first matmul gains), split-start PSUM groups (CoreSim
cannot model them; HW first_mm semantics uncertain).
"""

import numpy as np

import ml_dtypes

import concourse.tile as tile
from concourse import bacc, mybir
from concourse.bass_utils import run_bass_kernel_spmd
from contextlib import ExitStack

B = 4
N = 2048
D = 1024
P = 128
ET = D // P          # 8 contraction tiles over D
NQ = N // 2          # 1024 local queries per core
CHUNK = 512          # PSUM bank limit (512 fp32 outputs)
NCH = NQ // CHUNK    # 2 query chunks (phase 1)
KCH = N // CHUNK     # 4 key chunks (phase 2)
SCALE = 0.03125      # 1/sqrt(D), exact
WARMUP_MM = 6        # dummy matmuls to lift the PE HAM clock-gate early
F32 = mybir.dt.float32
BF16 = mybir.dt.bfloat16
EXP = mybir.ActivationFunctionType.Exp
ADD = mybir.AluOpType.add
MULT = mybir.AluOpType.mult
AXF = mybir.AxisListType.X

_CACHE = {}


def _build():
    mdt = BF16
    nc = bacc.Bacc(
        "TRN2",
        target_bir_lowering=False,
        debug=False,
        enable_asserts=False,
        num_devices=8,
    )
    # Per-core inputs. xq = x[b, local half].T ; xk2 = x[b, other half].T
    # (keys ordered local-first so the program is core-independent).
    xq_d = nc.dram_tensor("xq", [D, NQ], mdt, kind="ExternalInput")
    xk2_d = nc.dram_tensor("xk2", [D, NQ], mdt, kind="ExternalInput")
    a_d = nc.dram_tensor("A", [D, D], mdt, kind="ExternalInput")
    g_d = nc.dram_tensor("g", [P, ET], F32, kind="ExternalInput")
    wn_d = nc.dram_tensor("wn", [P, KCH, CHUNK], mdt, kind="ExternalInput")
    num_d = nc.dram_tensor("num", [P, KCH, ET], F32, kind="ExternalOutput")
    den_d = nc.dram_tensor("den", [P, KCH, ET], F32, kind="ExternalOutput")

    with tile.TileContext(nc) as tc, ExitStack() as ctx:
        const = ctx.enter_context(tc.tile_pool(name="const", bufs=1))
        ptp = ctx.enter_context(tc.tile_pool(name="pt", bufs=6))
        tmpp = ctx.enter_context(tc.tile_pool(name="tmp", bufs=3))
        workp = ctx.enter_context(
            tc.tile_pool(name="psum_work", bufs=7, space="PSUM")
        )
        wup = ctx.enter_context(
            tc.tile_pool(name="psum_wu", bufs=1, space="PSUM")
        )

        xq_sb = const.tile([P, ET, NQ], mdt)    # [p, f, n] : xT local cols
        xk2_sb = const.tile([P, ET, NQ], mdt)   # [p, f, n] : xT other cols
        a_sb = const.tile([P, ET, D], mdt)      # [p, f, e] : A tiles
        tt_sb = const.tile([P, ET, NQ], mdt)    # [p, e, n] : TT' tiles
        g_sb = const.tile([P, ET], F32)         # g[e] fold for the TT cast
        wn_sb = const.tile([P, KCH, CHUNK], mdt)  # wv[m] replicated per row
        num_sb = const.tile([P, KCH, ET], F32)
        den_sb = const.tile([P, KCH, ET], F32)
        wu_sb = const.tile([P, CHUNK], BF16)    # warmup operand (garbage ok)
        wu_sink = const.tile([P, 1], F32)

        # PE warm-up: keep TensorE busy from t~0 so the HAM clock-gate
        # lifts to 8/8 before the real matmuls start (they are DMA-gated),
        # and fill the DMA-paced holes of the first TT block below.
        # Operand contents are irrelevant.
        nc.gpsimd.memset(wu_sb[:], 0.0)
        wu_ps = wup.tile([P, CHUNK], F32)
        # per-f filler count topping up the real matmuls one (A[f], xq[f])
        # tile-pair arrival enables in the first block
        wpf = 1
        n_wu = WARMUP_MM + wpf * ET
        wu_iter = iter(range(n_wu))

        def warm(k):
            for _ in range(k):
                w = next(wu_iter, None)
                if w is None:
                    return
                nc.tensor.matmul(
                    wu_ps[:],
                    wu_sb[:, :P],
                    wu_sb[:],
                    start=(w == 0),
                    stop=(w == n_wu - 1),
                )

        warm(WARMUP_MM)

        # Input DMAs. Phase-1 block 0 (e 0-2) needs only A columns 0:384,
        # so those stream first alongside xq — this shrinks the critical
        # bytes gating the first matmul block. The rest of A arrives while
        # block 0 computes; wn is first read at the first phase-2 exp and
        # xk2 only gates key chunks 2-3 of phase 2, so they stream last.
        # DMA issue instructions cost ~750ns each on the issuing engine's
        # HWDGE ring; alternating Sync/Scalar rings halves the issue-cadence
        # latency (Scalar is idle until the first exp at ~45us).
        # f=0 is split finer: the very first matmul is gated on
        # A[:,0,0:128] + xq[:,0,0:512] (161KB) instead of the full 354KB
        # pair (the first transfers share DMA bandwidth round-robin with
        # everything issued behind them).
        E0 = 3 * P
        nc.sync.dma_start(a_sb[:, 0, :P], a_d[0:P, :P])
        nc.scalar.dma_start(xq_sb[:, 0, :CHUNK], xq_d[0:P, :CHUNK])
        nc.sync.dma_start(a_sb[:, 0, P:E0], a_d[0:P, P:E0])
        nc.scalar.dma_start(xq_sb[:, 0, CHUNK:], xq_d[0:P, CHUNK:])
        for f in range(1, ET):
            nc.sync.dma_start(a_sb[:, f, :E0], a_d[f * P:(f + 1) * P, :E0])
            nc.scalar.dma_start(xq_sb[:, f, :], xq_d[f * P:(f + 1) * P, :])
        nc.sync.dma_start(g_sb[:], g_d[:])
        # Keep these as per-f DMA instructions: separate instructions fan
        # out across parallel HW DMA queues (consolidating them into one
        # strided DMA measured ~6us slower end-to-end).
        for f in range(ET):
            eng = nc.sync if f % 2 == 0 else nc.scalar
            eng.dma_start(a_sb[:, f, E0:], a_d[f * P:(f + 1) * P, E0:])
        nc.sync.dma_start(wn_sb[:, 0:2, :], wn_d[:, 0:2, :])
        nc.scalar.dma_start(wn_sb[:, 2:4, :], wn_d[:, 2:4, :])
        for f in range(ET):
            eng = nc.sync if f % 2 == 0 else nc.scalar
            eng.dma_start(xk2_sb[:, f, :], xk2_d[f * P:(f + 1) * P, :])

        # Phase 1: TT'[e, n] = sum_f A[f, e-cols]^T . xT[f, n]  (+ g[e] in
        # the cast). e-blocks of 3 keep 6 PSUM accumulation groups open so
        # each arriving (A[f], xq[f]) DMA pair feeds 6 matmuls (less PE
        # starvation while inputs stream in).
        BLOCKS = [(0, 3), (3, 3), (6, 2)]
        for eb, (e0, blk) in enumerate(BLOCKS):
            pss = []
            for el in range(blk):
                row = [
                    workp.tile([P, CHUNK], F32,
                               name=f"tt_ps_{eb}_{el}_{j}", tag="ps")
                    for j in range(NCH)
                ]
                pss.append(row)
            for f in range(ET):
                # Block 0 runs j-outer: the three j=0 matmuls (first xq
                # half) front-load ~1.3us of cold-pace work, matching the
                # second xq half's later DMA arrival (it was the ~0.9us
                # stall at the start of the real stream).
                order = (
                    [(el, j) for j in range(NCH) for el in range(blk)]
                    if eb == 0 else
                    [(el, j) for el in range(blk) for j in range(NCH)]
                )
                for el, j in order:
                    e = e0 + el
                    nc.tensor.matmul(
                        pss[el][j][:],
                        a_sb[:, f, e * P:(e + 1) * P],
                        xq_sb[:, f, j * CHUNK:(j + 1) * CHUNK],
                        start=(f == 0),
                        stop=(f == ET - 1),
                    )
                if eb == 0:
                    # absorb the DMA-arrival pacing of the first block
                    warm(wpf)
            for el in range(blk):
                e = e0 + el
                for j in range(NCH):
                    nc.vector.tensor_scalar_add(
                        tt_sb[:, e, j * CHUNK:(j + 1) * CHUNK],
                        pss[el][j][:],
                        g_sb[:, e:e + 1],
                    )
            if eb == 0:
                warm(100)  # flush any leftover warmups
                nc.vector.tensor_copy(wu_sink[:], wu_ps[:, :1])

        # Phase 2: per (key-chunk c, query-slice s): S, exp, and the two
        # free-axis reductions on DVE (num's multiply on GpSimd). Key
        # chunks 0-1 (local xq keys) run first so the xk2 stream has until
        # ~mid-phase-2 to land.
        for c in range(KCH):
            xsrc = xq_sb if c < 2 else xk2_sb
            off = (c % 2) * CHUNK
            for s in range(ET):
                ps = workp.tile([P, CHUNK], F32, name=f"s_ps_{c}_{s}",
                                tag="ps")
                for e in range(ET):
                    nc.tensor.matmul(
                        ps[:],
                        tt_sb[:, e, s * P:(s + 1) * P],
                        xsrc[:, e, off:off + CHUNK],
                        start=(e == 0),
                        stop=(e == ET - 1),
                    )
                pt = ptp.tile([P, CHUNK], mdt, name=f"pt_{c}_{s}", tag="pt")
                # den rides the exp itself: ScalarE reduces its own output
                # into a per-partition scalar (softmax-denominator pattern),
                # halving the DVE load.
                nc.scalar.activation(
                    pt[:], ps[:], EXP, scale=SCALE,
                    accum_out=den_sb[:, c, s:s + 1],
                )
                tmp = tmpp.tile([P, CHUNK], F32, name=f"tmp_{c}_{s}",
                                tag="tmp")
                # GpSimd takes the num multiply except on the final two
                # groups, where the (slower, serialized) GpSimd would sit
                # on the exec tail; DVE has the slack there.
                meng = nc.vector if c == KCH - 1 and s >= ET - 2 else nc.gpsimd
                meng.tensor_tensor(tmp[:], pt[:], wn_sb[:, c, :], MULT)
                nc.vector.tensor_reduce(
                    num_sb[:, c, s:s + 1], tmp[:], AXF, ADD
                )

        # num in two pieces: chunks 0-2 stream out ~14us before the end;
        # only the tiny c=3 piece's completion sits inside the end barrier.
        nc.sync.dma_start(num_d[:, 0:KCH - 1, :], num_sb[:, 0:KCH - 1, :])
        nc.scalar.dma_start(den_d[:], den_sb[:])
        nc.sync.dma_start(num_d[:, KCH - 1:KCH, :],
                          num_sb[:, KCH - 1:KCH, :])

    nc.compile()
    return nc


def _get_nc():
    if "nc" not in _CACHE:
        _CACHE["nc"] = _build()
    return _CACHE["nc"]


def _prep(x, Wq, bq, Wk, bk, Wv, bv, Ww, bw):
    """Host-side sharding + weight folding -> per-core input maps."""
    x = np.asarray(x, dtype=np.float32)
    Wq = np.asarray(Wq, dtype=np.float32)
    bq = np.asarray(bq, dtype=np.float32)
    Wk = np.asarray(Wk, dtype=np.float32)
    Wv = np.asarray(Wv, dtype=np.float32)
    bv = np.asarray(bv, dtype=np.float32)
    Ww = np.asarray(Ww, dtype=np.float32)
    idt = ml_dtypes.bfloat16

    # Host-side weight folding (cheap: one 1024^3 sgemm + matvecs).
    A = np.ascontiguousarray(Wq.T @ Wk).astype(idt)  # [f, e]
    g = Wk.T @ bq                                    # key-side logit constant
    wv_eff = Wv.T @ Ww[0]                            # collapsed V @ Ww^T
    cvw = float(bv @ Ww[0])

    wv_all = x @ wv_eff + cvw                        # [B, N]
    g_arr = np.ascontiguousarray(g.reshape(ET, P).T.astype(np.float32))

    in_maps = []
    for core in range(8):
        b, h = divmod(core, 2)
        lo = np.arange(h * NQ, (h + 1) * NQ)
        hi = np.arange((1 - h) * NQ, (2 - h) * NQ)
        order = np.concatenate([lo, hi])             # keys: local half first
        wv_ord = wv_all[b][order].astype(idt)        # [N]
        wn = np.ascontiguousarray(
            np.broadcast_to(wv_ord.reshape(1, KCH, CHUNK), (P, KCH, CHUNK))
        )
        in_maps.append(
            {
                "xq": np.ascontiguousarray(x[b, lo].T.astype(idt)),
                "xk2": np.ascontiguousarray(x[b, hi].T.astype(idt)),
                "A": A,
                "g": g_arr,
                "wn": wn,
            }
        )
    return in_maps


def _gather(res, bw):
    bw = np.asarray(bw, dtype=np.float32)
    final = np.empty((B, N), dtype=np.float32)
    for core in range(8):
        b, h = divmod(core, 2)
        num = res.results[core]["num"].sum(axis=1)   # [P, ET]
        den = res.results[core]["den"].sum(axis=1)
        final[b, h * NQ:(h + 1) * NQ] = (num / den).T.reshape(NQ) + bw[0]
    return final


def kernel(x, Wq, bq, Wk, bk, Wv, bv, Ww, bw):
    nc = _get_nc()
    in_maps = _prep(x, Wq, bq, Wk, bk, Wv, bv, Ww, bw)
    res = run_bass_kernel_spmd(nc, in_maps, core_ids=list(range(8)))
    return _gather(res, bw)


def run_profiled(inputs, trace_cores=(0,)):
    """Run once with NTFF profiling; returns BassKernelResults."""
    nc = _get_nc()
    in_maps = _prep(**inputs)
    res = run_bass_kernel_spmd(
        nc, in_maps, core_ids=list(range(8)), trace=True,
        trace_cores=list(trace_cores),
    )
    return res
